# revision 11
# baseline (speedup 1.0000x reference)
"""LIF (leaky integrate-and-fire) forward kernel for Trainium2, 8 NeuronCores.

Recurrence (per element of [B, N], serial over T):
    v_t = DECAY * (v_{t-1} * (1 - s_{t-1})) + x_t      (REST = 0)
    s_t = (v_t > THRESHOLD)

v3.3 design: x is fed in fp16 (2 B/elem -> DMA floor ~47us/core).  The
carried state is u = DECAY * v * (v <= THR) in fp16, so the update is
v' = u + x.  The per-step spike indicator q = (v <= THR) * DECAY is 0 on
spike and ~0.2 otherwise; q doubles as the reset multiplier (u = q * v)
and as the packed-output source, so no Sign pass exists anywhere.

Two lane types over the per-core [128, 2048] slab:
  S-lane cols [0, C_S):    TensorE does the add in PSUM
      psv = I_f16^T x_f16 + I_f16^T u_f16      (2 matmuls, x one step early)
      v   = Identity(psv)     ScalarE -> fp16 SBUF
      q   = (v <= THR)*DECAY  DVE tensor_scalar (4x mode)
      u   = q * v             DVE tensor_tensor (2x mode)
  E-lane cols [C_S, F):    pure fp16 DVE
      v = u + x               DVE tensor_tensor (2x)
      q = (v <= THR)*DECAY    DVE tensor_scalar (4x)
      u = q * v               DVE tensor_tensor (2x)

Output: TensorE packs 8 steps of q into a PSUM byte-plane
(psum += 5*2^k I^T q_k ~= 2^k (1 - s_k)), one step deferred; ScalarE
drains it as int8 via out = -psum + 127 = S - 128 where S = sum 2^k s_k.
Host adds 128 and unpacks bits.
"""

import numpy as np

import concourse.bacc as bacc
import concourse.mybir as mybir
from concourse.tile import TileContext
from concourse.bass_utils import run_bass_kernel_spmd

T, B, N = 32, 128, 16384
N_CORES = 8
B_SH = B // N_CORES          # 16 batch rows per core
S = B_SH * N                 # 262144 elements per core per time step
P = 128                      # SBUF partitions
F = S // P                   # 2048 free-dim elements
DECAY = 0.2
THR = 0.3

C_S = 1536                   # PSUM-lane columns (multiple of 512)
C_E = F - C_S                # fp16 DVE-lane columns
N_SSUB = C_S // 512          # psv sub-lanes
CHUNK = 4                    # time steps per input DMA
GROUPS = T // 8              # byte-planes (8 steps packed per byte)

TRACE = False                # set True (e.g. from test.py) to capture a profile

_BUILT = {}


def _build_nc():
    nc = bacc.Bacc("TRN2", debug=False, num_devices=N_CORES)
    f16 = mybir.dt.float16
    f32 = mybir.dt.float32
    i8 = mybir.dt.int8
    Alu = mybir.AluOpType
    Act = mybir.ActivationFunctionType

    xs = nc.dram_tensor("xs", [P, T * F], f16, kind="ExternalInput").ap()
    wp_in = nc.dram_tensor("wpack", [P, 8 * P], f16, kind="ExternalInput").ap()
    ih_in = nc.dram_tensor("identh", [P, P], f16, kind="ExternalInput").ap()
    y = nc.dram_tensor("y", [P, GROUPS * F], i8, kind="ExternalOutput").ap()
    xr = xs.rearrange("p (t f) -> p t f", t=T)
    yr = y.rearrange("p (g f) -> p g f", g=GROUPS)

    E0 = C_S                 # E-lane column offset

    with TileContext(nc) as tc:
        with (
            tc.tile_pool(name="consts", bufs=1) as c_pool,
            tc.tile_pool(name="state", bufs=1) as st_pool,
            tc.tile_pool(name="xin", bufs=3) as xin_pool,
            tc.tile_pool(name="vs", bufs=3) as vs_pool,
            tc.tile_pool(name="qs", bufs=3) as qs_pool,
            tc.tile_pool(name="ve", bufs=3) as ve_pool,
            tc.tile_pool(name="qe", bufs=3) as qe_pool,
            tc.tile_pool(name="outs", bufs=2) as o_pool,
            tc.tile_pool(name="psv0", bufs=1, space="PSUM") as pv0_pool,
            tc.tile_pool(name="psv1", bufs=1, space="PSUM") as pv1_pool,
            tc.tile_pool(name="psv2", bufs=1, space="PSUM") as pv2_pool,
            tc.tile_pool(name="pack", bufs=1, space="PSUM") as pk_pool,
        ):
            zerob = nc.alloc_sbuf_tensor("const_zerob", [P, 1], f32).ap()
            nc.gpsimd.memset(zerob, 0.0)
            pos127 = nc.alloc_sbuf_tensor("const_pos127", [P, 1], f32).ap()
            nc.gpsimd.memset(pos127, 127.0)

            u_s = st_pool.tile([P, max(C_S, 1)], f16)    # S-lane state
            u_e = st_pool.tile([P, max(C_E, 1)], f16)    # E-lane state

            pv_pools = (pv0_pool, pv1_pool, pv2_pool)[:N_SSUB]

            def emit_pack(q_tiles, t_prev):
                # q_tiles: list of (ap, width) covering F columns of step
                # t_prev, in column order
                k = t_prev % 8
                wk = wsb[:, k * P:(k + 1) * P]
                col = 0
                for ap, wdt in q_tiles:
                    for r in range(0, wdt, 512):
                        nc.tensor.matmul(
                            out=pack_psum[:, col + r:col + r + 512], lhsT=wk,
                            rhs=ap[:, r:r + 512],
                            start=(k == 0), stop=(k == 7),
                        )
                    col += wdt

            def emit_drain(t_prev):
                # two halves so the next step's copies queue behind at most
                # half the drain on ScalarE
                g8 = t_prev // 8
                oi = o_pool.tile([P, F], i8, name="oi")
                h = F // 2
                nc.scalar.activation(
                    oi[:, :h], pack_psum[:, :h], Act.Identity,
                    bias=pos127, scale=-1.0)
                nc.scalar.activation(
                    oi[:, h:], pack_psum[:, h:], Act.Identity,
                    bias=pos127, scale=-1.0)
                nc.scalar.dma_start(out=yr[:, g8, :], in_=oi[:])

            # --- prologue: consts, first x in per-sublane slices, rest of
            # chunk 0.  identh first (needed by the warmup matmuls). ---
            identh = c_pool.tile([P, P], f16)
            nc.sync.dma_start(out=identh[:], in_=ih_in)
            xt = xin_pool.tile([P, CHUNK * F], f16, name="xt")
            for r in range(0, F, 512):
                nc.sync.dma_start(out=xt[:, r:r + 512], in_=xr[:, 0, r:r + 512])
            wsb = c_pool.tile([P, 8 * P], f16)
            nc.sync.dma_start(out=wsb[:], in_=wp_in)
            for jj in range(1, CHUNK):
                nc.sync.dma_start(
                    out=xt[:, jj * F:(jj + 1) * F], in_=xr[:, jj, :])

            # PE warmup: dummy matmuls during the first-x DMA wait so step 0
            # starts at a high PE p-state.  They write a psv-pool tile that
            # the real step-0 x-feed then overwrites (start=True).
            warm = pv0_pool.tile([P, 512], f32, name="warm")
            for _ in range(12):
                nc.tensor.matmul(
                    out=warm[:], lhsT=identh[:], rhs=wsb[:, :512],
                    start=True, stop=True)

            q_prev = None
            pack_psum = None
            psvs = None

            def x_feed(tn, xtile):
                """x-feed matmuls for step tn into fresh psv tiles."""
                jn = tn % CHUNK
                tiles = [pool.tile([P, 512], f32, name=f"ps{i}")
                         for i, pool in enumerate(pv_pools)]
                for i in range(N_SSUB):
                    nc.tensor.matmul(
                        out=tiles[i][:], lhsT=identh[:],
                        rhs=xtile[:, jn * F + i * 512:jn * F + (i + 1) * 512],
                        start=True, stop=(tn == 0))
                return tiles

            psvs = x_feed(0, xt)

            for t in range(T):
                j = t % CHUNK
                # (chunk tiles for t>0 are loaded at the bottom of the
                # previous iteration so their x-feed can be emitted early)
                xe = xt[:, j * F + E0:j * F + E0 + C_E]  # E-lane x

                if t % 8 == 0:
                    pack_psum = pk_pool.tile([P, F], f32, name="pk")

                # --- PE: pack of t-1, u-feed of t, x-feed of t+1 ---
                if q_prev is not None:
                    emit_pack(q_prev, t - 1)
                if t > 0:
                    for i in range(N_SSUB):
                        nc.tensor.matmul(
                            out=psvs[i][:], lhsT=identh[:],
                            rhs=u_s[:, i * 512:(i + 1) * 512],
                            start=False, stop=True)
                cur_psvs = psvs
                if t + 1 < T:
                    nxt = xt if (t + 1) % CHUNK != 0 else None
                    # next chunk tile isn't allocated yet when j==CHUNK-1;
                    # defer that x-feed to the top of the next iteration
                    if nxt is not None:
                        psvs = x_feed(t + 1, nxt)
                    else:
                        psvs = None

                # --- E-lane first on DVE (independent of ScalarE copies) ---
                if t == 0:
                    ve = xe
                else:
                    vet = ve_pool.tile([P, C_E], f16, name="ve")
                    nc.vector.tensor_tensor(
                        out=vet[:], in0=u_e[:], in1=xe, op=Alu.add)
                    ve = vet[:]
                qe = qe_pool.tile([P, C_E], f16, name="qe")
                nc.vector.tensor_scalar(
                    out=qe[:], in0=ve, scalar1=THR, scalar2=DECAY,
                    op0=Alu.is_le, op1=Alu.mult)
                nc.vector.tensor_tensor(
                    out=u_e[:], in0=qe[:], in1=ve, op=Alu.mult)

                # --- S-lane: ScalarE copy + DVE fast ops ---
                qss = []
                for i in range(N_SSUB):
                    vs = vs_pool.tile([P, 512], f16, name=f"vs{i}")
                    nc.scalar.activation(
                        vs[:], cur_psvs[i][:], Act.Identity, bias=zerob)
                    qs = qs_pool.tile([P, 512], f16, name=f"qs{i}")
                    nc.vector.tensor_scalar(
                        out=qs[:], in0=vs[:], scalar1=THR, scalar2=DECAY,
                        op0=Alu.is_le, op1=Alu.mult)
                    nc.vector.tensor_tensor(
                        out=u_s[:, i * 512:(i + 1) * 512], in0=qs[:],
                        in1=vs[:], op=Alu.mult)
                    qss.append((qs, 512))

                # group drain (once per 8 steps), ordered last on ScalarE
                if t > 0 and (t - 1) % 8 == 7:
                    emit_drain(t - 1)

                q_prev = qss + [(qe, C_E)]

                # if the next step starts a new chunk, allocate+load it now
                # and emit its x-feed so PE stays ahead
                if t + 1 < T and (t + 1) % CHUNK == 0:
                    xt = xin_pool.tile([P, CHUNK * F], f16, name="xt")
                    nc.sync.dma_start(
                        out=xt[:], in_=xr[:, t + 1:t + 1 + CHUNK, :])
                    psvs = x_feed(t + 1, xt)

            emit_pack(q_prev, T - 1)
            emit_drain(T - 1)
    nc.compile()
    return nc


LAST_RESULTS = None


def _make_consts():
    wp = np.zeros((P, 8 * P), dtype=np.float16)
    for k in range(8):
        wp[:, k * P:(k + 1) * P][np.arange(P), np.arange(P)] = \
            np.float16(5.0 * 2 ** k)
    ih = np.zeros((P, P), dtype=np.float16)
    ih[np.arange(P), np.arange(P)] = np.float16(1.0)
    return wp.view(np.uint16), ih.view(np.uint16)


def kernel(tx):
    global LAST_RESULTS
    tx = np.asarray(tx)
    assert tx.shape == (T, B, N) and tx.dtype == np.float32

    if "nc" not in _BUILT:
        _BUILT["nc"] = _build_nc()
    nc = _BUILT["nc"]

    wpack, ih = _make_consts()
    in_maps = []
    for c in range(N_CORES):
        xc = tx[:, c * B_SH:(c + 1) * B_SH, :].reshape(T, P, F)
        xc = np.ascontiguousarray(xc.transpose(1, 0, 2))     # [P, T, F]
        xh = xc.astype(np.float16).reshape(P, T * F)
        in_maps.append({
            "xs": xh.view(np.uint16),
            "wpack": wpack, "identh": ih,
        })

    res = run_bass_kernel_spmd(nc, in_maps, core_ids=list(range(N_CORES)),
                               trace=TRACE)
    LAST_RESULTS = res

    out = np.empty((T, B, N), dtype=np.float32)
    for c in range(N_CORES):
        yb = np.asarray(res.results[c]["y"]).astype(np.int16)  # [P, GROUPS*F]
        Bv = (yb + 128).astype(np.uint8).reshape(P, GROUPS, F)
        for g in range(GROUPS):
            for k in range(8):
                bits = (Bv[:, g, :] >> k) & 1          # [P, F]
                st = bits.reshape(B_SH, N).astype(np.float32)
                out[g * 8 + k, c * B_SH:(c + 1) * B_SH, :] = st
    return out
